# revision 19
# baseline (speedup 1.0000x reference)
"""MoE layer (top-2 of 8 experts, SwiGLU FFN) on 8 trn2 NeuronCores.

Strategy: expert parallelism, one expert per core. The host computes only the
top-2 *selection* (index lists) and performs dispatch/combine data movement
(gather tokens per expert / scatter-add partial outputs); all floating-point
math that produces output values — gate logits, top-2 softmax weights, the
SwiGLU FFN — runs on device.

v12 (675 -> ~655 us target) over the v4 bf16 baseline (745 us):
  - mixed-precision capacity split: each expert's C_LO smallest-gate-weight
    tokens run the whole FFN in fp8 e4m3 with DoubleRow matmuls (2 k-tiles
    per instruction, measured 2.0x bf16 PE throughput); the rest stay bf16.
    Low-gate-weight tokens carry small combine weights, so the fp8 error
    (~6.5% on that slice) adds only ~1.3e-2 to total rel err (measured on
    this input; gate 2e-2, final ~1.77e-2).
  - fp8 scale folding: w1*32 (silu activation scale 1/32), w3*8
    (h8 = silu(u) * psum_v stores 8*h directly, |8h| < 240 fp8 max),
    w2*64 (psum_y = 512*y; combine multiplies by wb/512).
  - overflow balancing: own capacity = mean load (2048), not max (2176).
    Heavy experts ship their excess pairs (smallest weights above the lo
    block) to other cores' fixed C_OV=128-column fp8 section, computed with
    a second (donor) weight stream and a donor gate rotation.
  - DMA ring discipline: one fast ring (sync) carries x in priority order
    (gate cols + first 1024 token cols first); gpsimd carries x8 + all fp8
    weights + f0's bf16 weights; scalar ring carries y writebacks. Spreading
    x across rings was measured SLOWER (queues share DMA engines
    round-robin, delaying the critical first transfer).
  - gate weights ride as the first 2E columns of xt (a separate tiny-
    descriptor gw DMA interleave-slowed the x transfer).
  - fp8 (lo/ov) tiles run FIRST within each f/dm block and psum depths are
    split h1:2 / h3:3 / y:3 so the in-order PE never waits on the vector
    engine's gating-transpose bursts (psum-bank WAR).
  - PE warm-up matmuls feed from a memset tile (no DMA dependency), keeping
    the p-state ramp going while x lands; stop-and-go starts were measured
    to leave the PE at a low clock (756 ns vs 216 ns per 512-col matmul).

Numerics (host-side numpy sim of the exact device arithmetic, this input):
bf16-only 4.4e-3; C_LO=448 sim 1.72e-2 -> HW 1.73e-2; +overflow ~1.77e-2.

Capping the capacity at 2048 by *dropping* overflow pairs was measured and
rejected: top-2 softmax gate weights on this input are never negligible
(min 0.034), so dropping pairs costs 2.7e-2 relative error — over the gate.
"""

import numpy as np

T, D, F, E = 8192, 1024, 4096, 8
NCORES = 8
P = 128
TOK_TILE = 512
C_LO = 448  # fp8 tokens per expert (smallest gate weights)
C_OV = 128  # overflow section: fp8 tokens of a DONOR expert (own cap = 2048)

_nc_cache: dict = {}


def _build(C: int, C_lo: int, C_ov: int):
    """Build + compile the per-core Bass program.

    C = total capacity (C_hi bf16 columns then C_lo fp8 columns)."""
    from contextlib import ExitStack

    import concourse.tile as tile
    from concourse import bacc, mybir
    from concourse.bass import ds

    f32 = mybir.dt.float32
    bf16 = mybir.dt.bfloat16
    f8 = mybir.dt.float8e4
    DR = mybir.MatmulPerfMode.DoubleRow
    KD, KF = D // P, F // P
    KH = KF // 2
    X = mybir.AxisListType.X
    Silu = mybir.ActivationFunctionType.Silu
    Tanh = mybir.ActivationFunctionType.Tanh
    Alu = mybir.AluOpType

    C_hi = C - C_lo

    nc = bacc.Bacc(
        "TRN2", target_bir_lowering=False, debug=False, num_devices=NCORES
    )
    # gate weights ride as the first 2E columns of xt: own rotation, then
    # donor rotation (dodges separate tiny-descriptor DMAs)
    XO = 2 * E
    GP = (-C_ov) % P  # zero padding so the ov gating tile is 128-wide
    xt = nc.dram_tensor("xt", [P, KD, XO + C + C_ov + GP], bf16, kind="ExternalInput")
    xt8 = nc.dram_tensor("xt8", [P, KD, C_lo + C_ov], f8, kind="ExternalInput")
    w1 = nc.dram_tensor("w1", [KF, P, KD, P], bf16, kind="ExternalInput")
    w3 = nc.dram_tensor("w3", [KF, P, KD, P], bf16, kind="ExternalInput")
    w2 = nc.dram_tensor("w2", [KD, P, KF, P], bf16, kind="ExternalInput")
    w18 = nc.dram_tensor("w18", [KF, P, KD, P], f8, kind="ExternalInput")
    w38 = nc.dram_tensor("w38", [KF, P, KD, P], f8, kind="ExternalInput")
    w28 = nc.dram_tensor("w28", [KD, P, KF, P], f8, kind="ExternalInput")
    w18o = nc.dram_tensor("w18o", [KF, P, KD, P], f8, kind="ExternalInput")
    w38o = nc.dram_tensor("w38o", [KF, P, KD, P], f8, kind="ExternalInput")
    w28o = nc.dram_tensor("w28o", [KD, P, KF, P], f8, kind="ExternalInput")
    yt = nc.dram_tensor("yt", [KD, P, C + C_ov], bf16, kind="ExternalOutput")
    # second F-half partials go to their own tensor; the host adds them.
    yt2 = nc.dram_tensor("yt2", [KD, P, C + C_ov], bf16, kind="ExternalOutput")

    # bf16 token tiles: 512s, remainder (multiple of 128) last
    tiles = []
    t0 = 0
    while t0 + TOK_TILE <= C_hi:
        tiles.append((t0, TOK_TILE))
        t0 += TOK_TILE
    if t0 < C_hi:
        tiles.append((t0, C_hi - t0))
    # fp8 token tiles (offsets within the lo section)
    tiles8 = []
    t0 = 0
    while t0 < C_lo:
        tt = min(TOK_TILE, C_lo - t0)
        tiles8.append((t0, tt))
        t0 += tt
    # gating tiles cover all C columns of xt (512-tiling, independent of
    # the hi/lo boundary — wb_lo is sliced out of wb_all afterwards)
    gtiles = []
    t0 = 0
    while t0 < C:
        tt = min(TOK_TILE, C - t0)
        gtiles.append((t0, tt, 0))
        t0 += tt
    gtiles.append((C, C_ov + GP, E))  # ov tokens gate on the donor rotation

    with ExitStack() as ctx:
        tc = ctx.enter_context(tile.TileContext(nc))
        const = ctx.enter_context(tc.tile_pool(name="const", bufs=1))
        xp = ctx.enter_context(tc.tile_pool(name="xp", bufs=1))
        wp = ctx.enter_context(tc.tile_pool(name="wp", bufs=4))
        wp8 = ctx.enter_context(tc.tile_pool(name="wp8", bufs=4))
        hp = ctx.enter_context(tc.tile_pool(name="hp", bufs=1))
        yp = ctx.enter_context(tc.tile_pool(name="yp", bufs=4))
        gp = ctx.enter_context(tc.tile_pool(name="gp", bufs=2))
        psA = ctx.enter_context(tc.tile_pool(name="psA", bufs=2, space="PSUM"))
        psA3 = ctx.enter_context(tc.tile_pool(name="psA3", bufs=3, space="PSUM"))
        psB = ctx.enter_context(tc.tile_pool(name="psB", bufs=3, space="PSUM"))

        # selector rows: picks partition 0 of the rhs in the broadcast matmul
        sel_sb = const.tile([32, P], bf16)
        nc.vector.memset(sel_sb[:], 0.0)
        nc.vector.memset(sel_sb[0:1, :], 1.0)

        # x in two half-loads: C/2-long runs per (partition, kd) keep DMA
        # descriptors >=2KB
        x_sb = xp.tile([P, KD, XO + C + C_ov + GP], bf16, tag="x", name="x")
        # priority chunks on the sync ring: the first gating tile unblocks
        # after just gate cols + 512 token cols; later chunks land while the
        # PE works. x8 rides the gpsimd ring in front of the f0 weights.
        XW = XO + C + C_ov + GP
        cuts = [0, XO + 512, XO + 1024, min(XO + 2048, XW), XW]
        for a, b in zip(cuts, cuts[1:]):
            if b > a:
                nc.sync.dma_start(x_sb[:, :, ds(a, b - a)], xt[:, :, ds(a, b - a)])
        x8_sb = xp.tile([P, KD, C_lo + C_ov], f8, tag="x8", name="x8")
        nc.gpsimd.dma_start(x8_sb[:], xt8[:, :, :])
        wb_all = xp.tile([P, C + C_ov + GP], f32, tag="wb_all", name="wba")
        wb_lo = xp.tile([P, C_lo + C_ov], f32, tag="wb_lo", name="wbl")

        # PE warm-up during the initial x DMA (sel_sb: memset-fed, no DMA dep)
        for wi in range(24):
            warm = psB.tile([P, P], f32, tag="y", name=f"warm_{wi}")
            nc.tensor.matmul(
                warm[:], sel_sb[:], sel_sb[:], start=True, stop=True,
            )

        wrt_tiles = []

        wA_pre = {}

        def load_wA(f, eng13=None):
            eng13 = eng13 or nc.sync
            w18_sb = wp8.tile([P, KD, P], f8, tag="w18", name=f"w18_{f}")
            nc.gpsimd.dma_start(w18_sb[:], w18[f])
            w38_sb = wp8.tile([P, KD, P], f8, tag="w38", name=f"w38_{f}")
            nc.gpsimd.dma_start(w38_sb[:], w38[f])
            w18o_sb = wp8.tile([P, KD, P], f8, tag="w18o", name=f"w18o_{f}")
            nc.gpsimd.dma_start(w18o_sb[:], w18o[f])
            w38o_sb = wp8.tile([P, KD, P], f8, tag="w38o", name=f"w38o_{f}")
            nc.gpsimd.dma_start(w38o_sb[:], w38o[f])
            w1_sb = wp.tile([P, KD, P], bf16, tag="w1", name=f"w1_{f}")
            eng13.dma_start(w1_sb[:], w1[f])
            w3_sb = wp.tile([P, KD, P], bf16, tag="w3", name=f"w3_{f}")
            eng13.dma_start(w3_sb[:], w3[f])
            return (w1_sb, w3_sb, w18_sb, w38_sb, w18o_sb, w38o_sb)

        wA_pre[0] = load_wA(0, eng13=nc.gpsimd)

        def emit_gating(t0, TT, goff=0):
            """Top-2 softmax weight of own expert for one token tile; leaves
            the transposed weight row in wrt_tiles for the selector matmul."""
            S = TT // P
            lt_ps = psB.tile([E, TT], f32, tag="y", name=f"lt_{t0}")
            for kd in range(KD):
                nc.tensor.matmul(
                    lt_ps[:],
                    x_sb[:, kd, ds(goff, E)],
                    x_sb[:, kd, ds(XO + t0, TT)],
                    start=(kd == 0),
                    stop=(kd == KD - 1),
                )
            lt32 = gp.tile([32, TT], f32, tag="lt32", name=f"lt32_{t0}")
            nc.vector.memset(lt32[:], 0.0)
            nc.vector.tensor_copy(lt32[0:E, :], lt_ps[:])
            lg = gp.tile([P, S, 32], f32, tag="lg", name=f"lg_{t0}")
            for s in range(S):
                for j in range(4):
                    nc.vector.transpose(
                        lg[ds(32 * j, 32), s],
                        lt32[:, ds(s * P + 32 * j, 32)],
                    )
            L = lg[:, :, 0:E]
            m1 = gp.tile([P, S, 1], f32, tag="m1", name=f"m1_{t0}")
            nc.vector.reduce_max(m1[:], L, axis=X)
            dd = gp.tile([P, S, E], f32, tag="d", name=f"d_{t0}")
            nc.vector.tensor_tensor(
                dd[:], L, m1[:].to_broadcast((P, S, E)), Alu.subtract
            )
            msk = gp.tile([P, S, E], f32, tag="msk", name=f"msk_{t0}")
            nc.vector.tensor_scalar(msk[:], dd[:], 0.0, None, Alu.is_ge)
            nc.vector.tensor_scalar(msk[:], msk[:], -100000.0, None, Alu.mult)
            nc.vector.tensor_add(msk[:], msk[:], dd[:])
            m2 = gp.tile([P, S, 1], f32, tag="m2", name=f"m2_{t0}")
            nc.vector.reduce_max(m2[:], msk[:], axis=X)
            # device m2 is RELATIVE (m2-m1, the mask adds dd): z = 2*dd0 - m2rel
            # = 2*l0 - m1 - m2true;  w = sigmoid(z) = 0.5 + 0.5*tanh(z/2)
            z = gp.tile([P, S, 1], f32, tag="z", name=f"z_{t0}")
            nc.vector.tensor_scalar(z[:], dd[:, :, 0:1], 2.0, None, Alu.mult)
            nc.vector.tensor_tensor(z[:], z[:], m2[:], Alu.subtract)
            th = gp.tile([P, S, 1], f32, tag="th", name=f"th_{t0}")
            nc.scalar.activation(th[:], z[:], Tanh, scale=0.5)
            wgt = gp.tile([P, S, 1], f32, tag=f"wgt{t0}", name=f"wgt_{t0}")
            nc.vector.tensor_scalar(wgt[:], th[:], 1.0, 0.5, Alu.add, Alu.mult)

            # wrt[32, TT]: row 0 carries the per-token weight, transposed
            wrt = gp.tile([32, TT], bf16, tag=f"wrt{t0}", name=f"wrt_{t0}")
            wcol = gp.tile([P, 32], bf16, tag="wcol", name=f"wcol_{t0}")
            for s in range(S):
                nc.vector.memset(wcol[:, 1:32], 0.0)
                nc.vector.tensor_copy(wcol[:, 0:1], wgt[:, s])
                for j in range(4):
                    nc.vector.transpose(
                        wrt[:, ds(s * P + 32 * j, 32)],
                        wcol[ds(32 * j, 32), :],
                    )
            wrt_tiles.append((t0, TT, wrt))

        def emit_sel():
            t0, TT, wrt = wrt_tiles.pop(0)
            wb_ps = psB.tile([P, TT], f32, tag="y", name=f"wbps_{t0}")
            nc.tensor.matmul(wb_ps[:], sel_sb[:], wrt[:], start=True, stop=True)
            nc.vector.tensor_copy(wb_all[:, ds(t0, TT)], wb_ps[:])

        # gating for the first two tiles leads; the rest interleave into
        # phase A so the PE is never waiting on an x-tile DMA
        gpend = list(gtiles)
        emit_gating(*gpend.pop(0))
        if gpend:
            emit_gating(*gpend.pop(0))

        for fh in range(2):
            # ---- phase A: h(F-half) = silu(w1.T x) * (w3.T x) ----
            h_sb = hp.tile([P, KH, C_hi], bf16, tag="h", name=f"h_{fh}")
            h8_sb = hp.tile([P, KH, C_lo + C_ov], f8, tag="h8", name=f"h8_{fh}")
            for fl in range(KH):
                f = fh * KH + fl
                if f in wA_pre:
                    w1_sb, w3_sb, w18_sb, w38_sb, w18o_sb, w38o_sb = wA_pre.pop(f)
                else:
                    w1_sb, w3_sb, w18_sb, w38_sb, w18o_sb, w38o_sb = load_wA(f)
                # ov sub-block: donor-expert fp8 tokens
                for t0, TT in [(0, C_ov)]:
                    uo = psA.tile([P, TT], f32, tag="h1", name=f"puo_{t0}_{f}")
                    vo = psA3.tile([P, TT], f32, tag="h3", name=f"pvo_{t0}_{f}")
                    for kd in range(0, KD, 2):
                        nc.tensor.matmul(
                            uo[:],
                            w18o_sb[:, kd:kd + 2, :],
                            x8_sb[:, kd:kd + 2, ds(C_lo + t0, TT)],
                            start=(kd == 0),
                            stop=(kd == KD - 2),
                            perf_mode=DR,
                        )
                    for kd in range(0, KD, 2):
                        nc.tensor.matmul(
                            vo[:],
                            w38o_sb[:, kd:kd + 2, :],
                            x8_sb[:, kd:kd + 2, ds(C_lo + t0, TT)],
                            start=(kd == 0),
                            stop=(kd == KD - 2),
                            perf_mode=DR,
                        )
                    s1o = gp.tile([P, TT], f32, tag="s1o", name=f"s1o_{t0}_{f}")
                    nc.scalar.activation(s1o[:], uo[:], Silu, scale=1.0 / 32.0)
                    nc.vector.tensor_tensor(
                        h8_sb[:, fl, ds(C_lo + t0, TT)], s1o[:], vo[:], Alu.mult
                    )
                # fp8 lo tiles: u = 32u in psum, silu scale 1/32; v = 8v;
                # h8 = silu(u) * 8v stored e4m3
                for t0, TT in tiles8:
                    u8 = psA.tile([P, TT], f32, tag="h1", name=f"pu8_{t0}_{f}")
                    v8 = psA3.tile([P, TT], f32, tag="h3", name=f"pv8_{t0}_{f}")
                    for kd in range(0, KD, 2):
                        nc.tensor.matmul(
                            u8[:],
                            w18_sb[:, kd:kd + 2, :],
                            x8_sb[:, kd:kd + 2, ds(t0, TT)],
                            start=(kd == 0),
                            stop=(kd == KD - 2),
                            perf_mode=DR,
                        )
                    for kd in range(0, KD, 2):
                        nc.tensor.matmul(
                            v8[:],
                            w38_sb[:, kd:kd + 2, :],
                            x8_sb[:, kd:kd + 2, ds(t0, TT)],
                            start=(kd == 0),
                            stop=(kd == KD - 2),
                            perf_mode=DR,
                        )
                    s18 = gp.tile([P, TT], f32, tag="s18", name=f"s18_{t0}_{f}")
                    nc.scalar.activation(s18[:], u8[:], Silu, scale=1.0 / 32.0)
                    nc.vector.tensor_tensor(
                        h8_sb[:, fl, ds(t0, TT)], s18[:], v8[:], Alu.mult
                    )
                for t0, TT in tiles:
                    h1 = psA.tile([P, TT], f32, tag="h1", name=f"ph1_{t0}_{f}")
                    h3 = psA3.tile([P, TT], f32, tag="h3", name=f"ph3_{t0}_{f}")
                    for kd in range(KD):
                        nc.tensor.matmul(
                            h1[:],
                            w1_sb[:, kd, :],
                            x_sb[:, kd, ds(XO + t0, TT)],
                            start=(kd == 0),
                            stop=(kd == KD - 1),
                        )
                    for kd in range(KD):
                        nc.tensor.matmul(
                            h3[:],
                            w3_sb[:, kd, :],
                            x_sb[:, kd, ds(XO + t0, TT)],
                            start=(kd == 0),
                            stop=(kd == KD - 1),
                        )
                    s1 = gp.tile([P, TT], f32, tag="s1", name=f"s1_{t0}_{f}")
                    nc.scalar.activation(s1[:], h1[:], Silu)
                    nc.vector.tensor_mul(h_sb[:, fl, ds(t0, TT)], s1[:], h3[:])
                # remaining gating chains, then selector matmuls, one per slot
                if fh == 0 and fl % 2 == 0:
                    if gpend:
                        emit_gating(*gpend.pop(0))
                    elif wrt_tiles:
                        emit_sel()

            if fh == 0:
                while gpend:
                    emit_gating(*gpend.pop(0))
                while wrt_tiles:
                    emit_sel()
                # wb/512 for the fp8 combine, one slice op
                nc.vector.tensor_scalar(
                    wb_lo[:], wb_all[:, ds(C_hi, C_lo + C_ov)],
                    1.0 / 512.0, None, Alu.mult,
                )

            # ---- phase B: yT(+=) (w2-half.T @ h) * wb ----
            for dm in range(KD):
                w2_sb = wp.tile([P, KH, P], bf16, tag="w2", name=f"w2_{fh}_{dm}")
                nc.sync.dma_start(w2_sb[:], w2[dm, :, ds(fh * KH, KH), :])
                w28_sb = wp8.tile([P, KH, P], f8, tag="w28", name=f"w28_{fh}_{dm}")
                nc.gpsimd.dma_start(w28_sb[:], w28[dm, :, ds(fh * KH, KH), :])
                w28o_sb = wp8.tile(
                    [P, KH, P], f8, tag="w28o", name=f"w28o_{fh}_{dm}"
                )
                nc.gpsimd.dma_start(w28o_sb[:], w28o[dm, :, ds(fh * KH, KH), :])
                dst = yt if fh == 0 else yt2
                # ov y chain
                for t0, TT in [(0, C_ov)]:
                    ypso = psB.tile(
                        [P, TT], f32, tag="y", name=f"yo_{t0}_{fh}_{dm}"
                    )
                    for fk in range(0, KH, 2):
                        nc.tensor.matmul(
                            ypso[:],
                            w28o_sb[:, fk:fk + 2, :],
                            h8_sb[:, fk:fk + 2, ds(C_lo + t0, TT)],
                            start=(fk == 0),
                            stop=(fk == KH - 2),
                            perf_mode=DR,
                        )
                    yo_sb = yp.tile(
                        [P, TT], bf16, tag="y_sb", name=f"yosb_{t0}_{fh}_{dm}"
                    )
                    nc.vector.tensor_mul(
                        yo_sb[:], ypso[:], wb_lo[:, ds(C_lo + t0, TT)]
                    )
                    nc.scalar.dma_start(dst[dm, :, ds(C + t0, TT)], yo_sb[:])
                # fp8 lo tiles: psum = 512*y; combine with wb/512
                for t0, TT in tiles8:
                    yps8 = psB.tile(
                        [P, TT], f32, tag="y", name=f"y8_{t0}_{fh}_{dm}"
                    )
                    for fk in range(0, KH, 2):
                        nc.tensor.matmul(
                            yps8[:],
                            w28_sb[:, fk:fk + 2, :],
                            h8_sb[:, fk:fk + 2, ds(t0, TT)],
                            start=(fk == 0),
                            stop=(fk == KH - 2),
                            perf_mode=DR,
                        )
                    y8_sb = yp.tile(
                        [P, TT], bf16, tag="y_sb", name=f"y8sb_{t0}_{fh}_{dm}"
                    )
                    nc.vector.tensor_mul(
                        y8_sb[:], yps8[:], wb_lo[:, ds(t0, TT)]
                    )
                    nc.scalar.dma_start(dst[dm, :, ds(C_hi + t0, TT)], y8_sb[:])

                for t0, TT in tiles:
                    yps = psB.tile([P, TT], f32, tag="y", name=f"y_{t0}_{fh}_{dm}")
                    for fk in range(KH):
                        nc.tensor.matmul(
                            yps[:],
                            w2_sb[:, fk, :],
                            h_sb[:, fk, ds(t0, TT)],
                            start=(fk == 0),
                            stop=(fk == KH - 1),
                        )
                    y_sb = yp.tile(
                        [P, TT], bf16, tag="y_sb", name=f"ysb_{t0}_{fh}_{dm}"
                    )
                    nc.vector.tensor_mul(y_sb[:], yps[:], wb_all[:, ds(t0, TT)])
                    nc.scalar.dma_start(dst[dm, :, ds(t0, TT)], y_sb[:])
    nc.compile()
    return nc


def _route(x: np.ndarray, gw: np.ndarray):
    """Top-2 expert selection + combine weights (host; softmax weights are
    used only for ORDERING tokens by weight — output values use the
    device-computed gate)."""
    logits = x @ gw
    n = x.shape[0]
    top1 = np.argmax(logits, axis=1)
    l2 = logits.copy()
    l2[np.arange(n), top1] = -np.inf
    top2 = np.argmax(l2, axis=1)
    m = np.maximum(logits[np.arange(n), top1], logits[np.arange(n), top2])
    e1 = np.exp(logits[np.arange(n), top1] - m)
    e2 = np.exp(logits[np.arange(n), top2] - m)
    cw1 = e1 / (e1 + e2)
    cw2 = e2 / (e1 + e2)
    idx, cws = [], []
    for e in range(gw.shape[1]):
        i1 = np.nonzero(top1 == e)[0]
        i2 = np.nonzero(top2 == e)[0]
        ii = np.concatenate([i1, i2])
        ww = np.concatenate([cw1[i1], cw2[i2]])
        order = np.argsort(ww)
        idx.append(ii[order].astype(np.int64))
        cws.append(ww[order])
    return idx


def _shuffle_w13(w: np.ndarray):
    # [D, F] -> [KF, P, KD, P] partition-major blocks
    KD, KF = D // P, F // P
    return np.ascontiguousarray(w.reshape(KD, P, KF, P).transpose(2, 1, 0, 3))


def _shuffle_w2(w: np.ndarray):
    # [F, D] -> [KD, P, KF, P]
    KD, KF = D // P, F // P
    return np.ascontiguousarray(w.reshape(KF, P, KD, P).transpose(2, 1, 0, 3))


def kernel(x, gate_w, w1, w2, w3, _trace=False, _trace_cores=None, _result_box=None):
    import ml_dtypes
    from concourse.bass_utils import run_bass_kernel_spmd

    bf16 = ml_dtypes.bfloat16
    f8 = ml_dtypes.float8_e4m3
    KD = D // P

    x = np.ascontiguousarray(np.asarray(x, dtype=np.float32))
    gw = np.ascontiguousarray(np.asarray(gate_w, dtype=np.float32))
    w1 = np.ascontiguousarray(np.asarray(w1, dtype=np.float32))
    w2 = np.ascontiguousarray(np.asarray(w2, dtype=np.float32))
    w3 = np.ascontiguousarray(np.asarray(w3, dtype=np.float32))
    assert x.shape == (T, D) and gw.shape == (D, E), (x.shape, gw.shape)
    assert w1.shape == (E, D, F) and w3.shape == (E, D, F), (w1.shape,)
    assert w2.shape == (E, F, D), (w2.shape,)

    idx = _route(x, gw)
    maxn = max(len(i) for i in idx)
    # own capacity: the mean load (padded); heavy experts ship their excess
    # (smallest-gate-weight pairs above the own-lo block) to other cores'
    # C_OV-wide fp8 overflow section. Falls back to max-load capacity if the
    # shipments don't fit.
    C = -(-(T * 2 // E) // P) * P
    ship = {e: len(idx[e]) - C for e in range(E) if len(idx[e]) > C}
    if sum(ship.values()) > E * C_OV:
        C = max(P, -(-maxn // P) * P)
        ship = {}
    C_lo = C_LO if all(len(i) > C_LO + ship.get(e, 0) for e, i in enumerate(idx)) else 0
    C_hi = C - C_lo
    # donor assignment: shipments split into <=C_OV pieces, each piece to
    # any core's ov slot (SPMD time is C+C_OV everywhere, so a donor may
    # host overflow too). assign[c] = (donor, start_within_shipment, n)
    assign = {}
    free = [c for c in range(E) if c not in ship] + sorted(ship)
    for d in sorted(ship, key=lambda e: -ship[e]):
        off = 0
        while off < ship[d]:
            n = min(C_OV, ship[d] - off)
            c = free.pop(0)
            assign[c] = (d, off, n)
            off += n
    for c in range(E):
        if c not in assign:
            assign[c] = (c, 0, 0)  # dummy: own weights, zero ov columns

    if (C, C_lo) not in _nc_cache:
        _nc_cache[(C, C_lo)] = _build(C, C_lo, C_OV)
    nc = _nc_cache[(C, C_lo)]

    rot = np.arange(E)
    XO = 2 * E
    # per-core pair lists: own-lo (C_lo smallest), shipped-out, own-hi
    los, his, ovs = [], [], []
    for e in range(E):
        k = ship.get(e, 0)
        los.append(idx[e][:C_lo])
        ovs.append(idx[e][C_lo:C_lo + k])      # shipped OUT of this expert
        his.append(idx[e][C_lo + k:])
    w18a = [_shuffle_w13((32.0 * w1[e]).astype(f8)) for e in range(E)]
    w38a = [_shuffle_w13((8.0 * w3[e]).astype(f8)) for e in range(E)]
    w28a = [_shuffle_w2((64.0 * w2[e]).astype(f8)) for e in range(E)]
    in_maps = []
    for e in range(E):
        lo, hi = los[e], his[e]
        n_hi = len(hi)
        d, ov_off, k = assign[e]
        ovt = ovs[d][ov_off:ov_off + k]
        xt = np.zeros((P, KD, XO + C + C_OV + ((-C_OV) % P)), bf16)
        gwr = np.ascontiguousarray(gw[:, (rot + e) % E]).astype(bf16)
        xt[:, :, :E] = gwr.reshape(KD, P, E).transpose(1, 0, 2)
        gwd = np.ascontiguousarray(gw[:, (rot + d) % E]).astype(bf16)
        xt[:, :, E:XO] = gwd.reshape(KD, P, E).transpose(1, 0, 2)
        xt[:, :, XO:XO + n_hi] = (
            x[hi].astype(bf16).reshape(n_hi, KD, P).transpose(2, 1, 0)
        )
        xt[:, :, XO + C_hi:XO + C] = (
            x[lo].astype(bf16).reshape(C_lo, KD, P).transpose(2, 1, 0)
        )
        if k:
            xt[:, :, XO + C:XO + C + k] = (
                x[ovt].astype(bf16).reshape(k, KD, P).transpose(2, 1, 0)
            )
        xt8 = np.zeros((P, KD, C_lo + C_OV), f8)
        xt8[:, :, :C_lo] = (
            x[lo].astype(f8).reshape(C_lo, KD, P).transpose(2, 1, 0)
        )
        if k:
            xt8[:, :, C_lo:C_lo + k] = (
                x[ovt].astype(f8).reshape(k, KD, P).transpose(2, 1, 0)
            )
        in_maps.append(
            {
                "xt": xt,
                "xt8": xt8,
                "w1": _shuffle_w13(w1[e].astype(bf16)),
                "w3": _shuffle_w13(w3[e].astype(bf16)),
                "w2": _shuffle_w2(w2[e].astype(bf16)),
                "w18": w18a[e],
                "w38": w38a[e],
                "w28": w28a[e],
                "w18o": w18a[d],
                "w38o": w38a[d],
                "w28o": w28a[d],
            }
        )

    res = run_bass_kernel_spmd(
        nc,
        in_maps,
        core_ids=list(range(NCORES)),
        trace=_trace,
        trace_cores=_trace_cores,
    )
    if _result_box is not None:
        _result_box.append(res)

    out = np.zeros((T, D), np.float32)
    for e in range(E):
        lo, hi = los[e], his[e]
        n_hi = len(hi)
        d, ov_off, k = assign[e]
        ovt = ovs[d][ov_off:ov_off + k]
        yt = np.asarray(res.results[e]["yt"])        # [KD, P, C + C_OV] bf16
        yt2 = np.asarray(res.results[e]["yt2"])
        ysum = yt.astype(np.float32) + yt2.astype(np.float32)
        out[hi] += ysum[:, :, :n_hi].reshape(D, n_hi).T
        if C_lo:
            out[lo] += ysum[:, :, C_hi:C].reshape(D, C_lo).T
        if k:
            out[ovt] += ysum[:, :, C:C + k].reshape(D, k).T
    return out


# revision 20
# speedup vs baseline: 1.1981x; 1.1981x over previous
"""MoE layer (top-2 of 8 experts, SwiGLU FFN) on 8 trn2 NeuronCores.

Strategy: expert parallelism, one expert per core. The host computes only the
top-2 *selection* (index lists) and performs dispatch/combine data movement
(gather tokens per expert / scatter-add partial outputs); all floating-point
math that produces output values — gate logits, top-2 softmax weights, the
SwiGLU FFN — runs on device.

v12 (675 -> ~655 us target) over the v4 bf16 baseline (745 us):
  - mixed-precision capacity split: each expert's C_LO smallest-gate-weight
    tokens run the whole FFN in fp8 e4m3 with DoubleRow matmuls (2 k-tiles
    per instruction, measured 2.0x bf16 PE throughput); the rest stay bf16.
    Low-gate-weight tokens carry small combine weights, so the fp8 error
    (~6.5% on that slice) adds only ~1.3e-2 to total rel err (measured on
    this input; gate 2e-2, final ~1.77e-2).
  - fp8 scale folding: w1*32 (silu activation scale 1/32), w3*8
    (h8 = silu(u) * psum_v stores 8*h directly, |8h| < 240 fp8 max),
    w2*64 (psum_y = 512*y; combine multiplies by wb/512).
  - overflow balancing: own capacity = mean load (2048), not max (2176).
    Heavy experts ship their excess pairs (smallest weights above the lo
    block) to other cores' fixed C_OV=128-column fp8 section, computed with
    a second (donor) weight stream and a donor gate rotation.
  - DMA ring discipline: one fast ring (sync) carries x in priority order
    (gate cols + first 1024 token cols first); gpsimd carries x8 + all fp8
    weights + f0's bf16 weights; scalar ring carries y writebacks. Spreading
    x across rings was measured SLOWER (queues share DMA engines
    round-robin, delaying the critical first transfer).
  - gate weights ride as the first 2E columns of xt (a separate tiny-
    descriptor gw DMA interleave-slowed the x transfer).
  - fp8 (lo/ov) tiles run FIRST within each f/dm block and psum depths are
    split h1:2 / h3:3 / y:3 so the in-order PE never waits on the vector
    engine's gating-transpose bursts (psum-bank WAR).
  - PE warm-up matmuls feed from a memset tile (no DMA dependency), keeping
    the p-state ramp going while x lands; stop-and-go starts were measured
    to leave the PE at a low clock (756 ns vs 216 ns per 512-col matmul).

Numerics (host-side numpy sim of the exact device arithmetic, this input):
bf16-only 4.4e-3; C_LO=448 sim 1.72e-2 -> HW 1.73e-2; +overflow ~1.77e-2.

Capping the capacity at 2048 by *dropping* overflow pairs was measured and
rejected: top-2 softmax gate weights on this input are never negligible
(min 0.034), so dropping pairs costs 2.7e-2 relative error — over the gate.
"""

import numpy as np

T, D, F, E = 8192, 1024, 4096, 8
NCORES = 8
P = 128
TOK_TILE = 512
C_LO = 448  # fp8 tokens per expert (smallest gate weights)
C_OV = 128  # overflow section: fp8 tokens of a DONOR expert (own cap = 2048)

_nc_cache: dict = {}


def _build(C: int, C_lo: int, C_ov: int):
    """Build + compile the per-core Bass program.

    C = total capacity (C_hi bf16 columns then C_lo fp8 columns)."""
    from contextlib import ExitStack

    import concourse.tile as tile
    from concourse import bacc, mybir
    from concourse.bass import ds

    f32 = mybir.dt.float32
    bf16 = mybir.dt.bfloat16
    f8 = mybir.dt.float8e4
    DR = mybir.MatmulPerfMode.DoubleRow
    KD, KF = D // P, F // P
    KH = KF // 2
    X = mybir.AxisListType.X
    Silu = mybir.ActivationFunctionType.Silu
    Tanh = mybir.ActivationFunctionType.Tanh
    Alu = mybir.AluOpType

    C_hi = C - C_lo

    nc = bacc.Bacc(
        "TRN2", target_bir_lowering=False, debug=False, num_devices=NCORES
    )
    # gate weights ride as the first 2E columns of xt: own rotation, then
    # donor rotation (dodges separate tiny-descriptor DMAs)
    XO = 2 * E
    GP = (-C_ov) % P  # zero padding so the ov gating tile is 128-wide
    xt = nc.dram_tensor("xt", [P, KD, XO + C + C_ov + GP], bf16, kind="ExternalInput")
    xt8 = nc.dram_tensor("xt8", [P, KD, C_lo + C_ov], f8, kind="ExternalInput")
    w1 = nc.dram_tensor("w1", [KF, P, KD, P], bf16, kind="ExternalInput")
    w3 = nc.dram_tensor("w3", [KF, P, KD, P], bf16, kind="ExternalInput")
    w2 = nc.dram_tensor("w2", [KD, P, KF, P], bf16, kind="ExternalInput")
    w18 = nc.dram_tensor("w18", [KF, P, KD, P], f8, kind="ExternalInput")
    w38 = nc.dram_tensor("w38", [KF, P, KD, P], f8, kind="ExternalInput")
    w28 = nc.dram_tensor("w28", [KD, P, KF, P], f8, kind="ExternalInput")
    w18o = nc.dram_tensor("w18o", [KF, P, KD, P], f8, kind="ExternalInput")
    w38o = nc.dram_tensor("w38o", [KF, P, KD, P], f8, kind="ExternalInput")
    w28o = nc.dram_tensor("w28o", [KD, P, KF, P], f8, kind="ExternalInput")
    yt = nc.dram_tensor("yt", [KD, P, C + C_ov], bf16, kind="ExternalOutput")
    # second F-half partials go to their own tensor; the host adds them.
    yt2 = nc.dram_tensor("yt2", [KD, P, C + C_ov], bf16, kind="ExternalOutput")

    # bf16 token tiles: 512s, remainder (multiple of 128) last
    tiles = []
    t0 = 0
    while t0 + TOK_TILE <= C_hi:
        tiles.append((t0, TOK_TILE))
        t0 += TOK_TILE
    if t0 < C_hi:
        tiles.append((t0, C_hi - t0))
    # fp8 token tiles (offsets within the lo section)
    tiles8 = []
    t0 = 0
    while t0 < C_lo:
        tt = min(TOK_TILE, C_lo - t0)
        tiles8.append((t0, tt))
        t0 += tt
    # gating tiles cover all C columns of xt (512-tiling, independent of
    # the hi/lo boundary — wb_lo is sliced out of wb_all afterwards)
    gtiles = []
    t0 = 0
    while t0 < C:
        tt = min(TOK_TILE, C - t0)
        gtiles.append((t0, tt, 0))
        t0 += tt
    gtiles.append((C, C_ov + GP, E))  # ov tokens gate on the donor rotation

    with ExitStack() as ctx:
        tc = ctx.enter_context(tile.TileContext(nc))
        const = ctx.enter_context(tc.tile_pool(name="const", bufs=1))
        xp = ctx.enter_context(tc.tile_pool(name="xp", bufs=1))
        wp = ctx.enter_context(tc.tile_pool(name="wp", bufs=4))
        wp8 = ctx.enter_context(tc.tile_pool(name="wp8", bufs=4))
        hp = ctx.enter_context(tc.tile_pool(name="hp", bufs=1))
        yp = ctx.enter_context(tc.tile_pool(name="yp", bufs=4))
        gp = ctx.enter_context(tc.tile_pool(name="gp", bufs=2))
        psA = ctx.enter_context(tc.tile_pool(name="psA", bufs=2, space="PSUM"))
        psA3 = ctx.enter_context(tc.tile_pool(name="psA3", bufs=3, space="PSUM"))
        psB = ctx.enter_context(tc.tile_pool(name="psB", bufs=3, space="PSUM"))

        # selector rows: picks partition 0 of the rhs in the broadcast matmul
        sel_sb = const.tile([32, P], bf16)
        nc.vector.memset(sel_sb[:], 0.0)
        nc.vector.memset(sel_sb[0:1, :], 1.0)

        # x in two half-loads: C/2-long runs per (partition, kd) keep DMA
        # descriptors >=2KB
        x_sb = xp.tile([P, KD, XO + C + C_ov + GP], bf16, tag="x", name="x")
        CH = XO + 1024
        # priority order on the sync ring: gate cols + first-gating chunk,
        # then the rest; x8 rides the gpsimd ring in front of the f0 weights
        nc.sync.dma_start(x_sb[:, :, ds(0, CH)], xt[:, :, ds(0, CH)])
        XW = XO + C + C_ov + GP
        nc.sync.dma_start(x_sb[:, :, ds(CH, XW - CH)], xt[:, :, ds(CH, XW - CH)])
        x8_sb = xp.tile([P, KD, C_lo + C_ov], f8, tag="x8", name="x8")
        nc.gpsimd.dma_start(x8_sb[:], xt8[:, :, :])
        wb_all = xp.tile([P, C + C_ov + GP], f32, tag="wb_all", name="wba")
        wb_lo = xp.tile([P, C_lo + C_ov], f32, tag="wb_lo", name="wbl")

        # PE warm-up during the initial x DMA (sel_sb: memset-fed, no DMA dep)
        for wi in range(24):
            warm = psB.tile([P, P], f32, tag="y", name=f"warm_{wi}")
            nc.tensor.matmul(
                warm[:], sel_sb[:], sel_sb[:], start=True, stop=True,
            )

        wrt_tiles = []

        wA_pre = {}

        def load_wA(f, eng13=None):
            eng13 = eng13 or nc.sync
            w18_sb = wp8.tile([P, KD, P], f8, tag="w18", name=f"w18_{f}")
            nc.gpsimd.dma_start(w18_sb[:], w18[f])
            w38_sb = wp8.tile([P, KD, P], f8, tag="w38", name=f"w38_{f}")
            nc.gpsimd.dma_start(w38_sb[:], w38[f])
            w18o_sb = wp8.tile([P, KD, P], f8, tag="w18o", name=f"w18o_{f}")
            nc.gpsimd.dma_start(w18o_sb[:], w18o[f])
            w38o_sb = wp8.tile([P, KD, P], f8, tag="w38o", name=f"w38o_{f}")
            nc.gpsimd.dma_start(w38o_sb[:], w38o[f])
            w1_sb = wp.tile([P, KD, P], bf16, tag="w1", name=f"w1_{f}")
            eng13.dma_start(w1_sb[:], w1[f])
            w3_sb = wp.tile([P, KD, P], bf16, tag="w3", name=f"w3_{f}")
            eng13.dma_start(w3_sb[:], w3[f])
            return (w1_sb, w3_sb, w18_sb, w38_sb, w18o_sb, w38o_sb)

        wA_pre[0] = load_wA(0, eng13=nc.gpsimd)

        def emit_gating(t0, TT, goff=0):
            """Top-2 softmax weight of own expert for one token tile; leaves
            the transposed weight row in wrt_tiles for the selector matmul."""
            S = TT // P
            lt_ps = psB.tile([E, TT], f32, tag="y", name=f"lt_{t0}")
            for kd in range(KD):
                nc.tensor.matmul(
                    lt_ps[:],
                    x_sb[:, kd, ds(goff, E)],
                    x_sb[:, kd, ds(XO + t0, TT)],
                    start=(kd == 0),
                    stop=(kd == KD - 1),
                )
            lt32 = gp.tile([32, TT], f32, tag="lt32", name=f"lt32_{t0}")
            nc.vector.memset(lt32[:], 0.0)
            nc.vector.tensor_copy(lt32[0:E, :], lt_ps[:])
            lg = gp.tile([P, S, 32], f32, tag="lg", name=f"lg_{t0}")
            for s in range(S):
                for j in range(4):
                    nc.vector.transpose(
                        lg[ds(32 * j, 32), s],
                        lt32[:, ds(s * P + 32 * j, 32)],
                    )
            L = lg[:, :, 0:E]
            m1 = gp.tile([P, S, 1], f32, tag="m1", name=f"m1_{t0}")
            nc.vector.reduce_max(m1[:], L, axis=X)
            dd = gp.tile([P, S, E], f32, tag="d", name=f"d_{t0}")
            nc.vector.tensor_tensor(
                dd[:], L, m1[:].to_broadcast((P, S, E)), Alu.subtract
            )
            msk = gp.tile([P, S, E], f32, tag="msk", name=f"msk_{t0}")
            nc.vector.tensor_scalar(msk[:], dd[:], 0.0, None, Alu.is_ge)
            nc.vector.tensor_scalar(msk[:], msk[:], -100000.0, None, Alu.mult)
            nc.vector.tensor_add(msk[:], msk[:], dd[:])
            m2 = gp.tile([P, S, 1], f32, tag="m2", name=f"m2_{t0}")
            nc.vector.reduce_max(m2[:], msk[:], axis=X)
            # device m2 is RELATIVE (m2-m1, the mask adds dd): z = 2*dd0 - m2rel
            # = 2*l0 - m1 - m2true;  w = sigmoid(z) = 0.5 + 0.5*tanh(z/2)
            z = gp.tile([P, S, 1], f32, tag="z", name=f"z_{t0}")
            nc.vector.tensor_scalar(z[:], dd[:, :, 0:1], 2.0, None, Alu.mult)
            nc.vector.tensor_tensor(z[:], z[:], m2[:], Alu.subtract)
            th = gp.tile([P, S, 1], f32, tag="th", name=f"th_{t0}")
            nc.scalar.activation(th[:], z[:], Tanh, scale=0.5)
            wgt = gp.tile([P, S, 1], f32, tag=f"wgt{t0}", name=f"wgt_{t0}")
            nc.vector.tensor_scalar(wgt[:], th[:], 1.0, 0.5, Alu.add, Alu.mult)

            # wrt[32, TT]: row 0 carries the per-token weight, transposed
            wrt = gp.tile([32, TT], bf16, tag=f"wrt{t0}", name=f"wrt_{t0}")
            wcol = gp.tile([P, 32], bf16, tag="wcol", name=f"wcol_{t0}")
            for s in range(S):
                nc.vector.memset(wcol[:, 1:32], 0.0)
                nc.vector.tensor_copy(wcol[:, 0:1], wgt[:, s])
                for j in range(4):
                    nc.vector.transpose(
                        wrt[:, ds(s * P + 32 * j, 32)],
                        wcol[ds(32 * j, 32), :],
                    )
            wrt_tiles.append((t0, TT, wrt))

        def emit_sel():
            t0, TT, wrt = wrt_tiles.pop(0)
            wb_ps = psB.tile([P, TT], f32, tag="y", name=f"wbps_{t0}")
            nc.tensor.matmul(wb_ps[:], sel_sb[:], wrt[:], start=True, stop=True)
            nc.vector.tensor_copy(wb_all[:, ds(t0, TT)], wb_ps[:])

        # gating for the first two tiles leads; the rest interleave into
        # phase A so the PE is never waiting on an x-tile DMA
        gpend = list(gtiles)
        emit_gating(*gpend.pop(0))
        if gpend:
            emit_gating(*gpend.pop(0))

        for fh in range(2):
            # ---- phase A: h(F-half) = silu(w1.T x) * (w3.T x) ----
            h_sb = hp.tile([P, KH, C_hi], bf16, tag="h", name=f"h_{fh}")
            h8_sb = hp.tile([P, KH, C_lo + C_ov], f8, tag="h8", name=f"h8_{fh}")
            for fl in range(KH):
                f = fh * KH + fl
                if f in wA_pre:
                    w1_sb, w3_sb, w18_sb, w38_sb, w18o_sb, w38o_sb = wA_pre.pop(f)
                else:
                    w1_sb, w3_sb, w18_sb, w38_sb, w18o_sb, w38o_sb = load_wA(f)
                # ov sub-block: donor-expert fp8 tokens
                for t0, TT in [(0, C_ov)]:
                    uo = psA.tile([P, TT], f32, tag="h1", name=f"puo_{t0}_{f}")
                    vo = psA3.tile([P, TT], f32, tag="h3", name=f"pvo_{t0}_{f}")
                    for kd in range(0, KD, 2):
                        nc.tensor.matmul(
                            uo[:],
                            w18o_sb[:, kd:kd + 2, :],
                            x8_sb[:, kd:kd + 2, ds(C_lo + t0, TT)],
                            start=(kd == 0),
                            stop=(kd == KD - 2),
                            perf_mode=DR,
                        )
                    for kd in range(0, KD, 2):
                        nc.tensor.matmul(
                            vo[:],
                            w38o_sb[:, kd:kd + 2, :],
                            x8_sb[:, kd:kd + 2, ds(C_lo + t0, TT)],
                            start=(kd == 0),
                            stop=(kd == KD - 2),
                            perf_mode=DR,
                        )
                    s1o = gp.tile([P, TT], f32, tag="s1o", name=f"s1o_{t0}_{f}")
                    nc.scalar.activation(s1o[:], uo[:], Silu, scale=1.0 / 32.0)
                    nc.vector.tensor_tensor(
                        h8_sb[:, fl, ds(C_lo + t0, TT)], s1o[:], vo[:], Alu.mult
                    )
                # fp8 lo tiles: u = 32u in psum, silu scale 1/32; v = 8v;
                # h8 = silu(u) * 8v stored e4m3
                for t0, TT in tiles8:
                    u8 = psA.tile([P, TT], f32, tag="h1", name=f"pu8_{t0}_{f}")
                    v8 = psA3.tile([P, TT], f32, tag="h3", name=f"pv8_{t0}_{f}")
                    for kd in range(0, KD, 2):
                        nc.tensor.matmul(
                            u8[:],
                            w18_sb[:, kd:kd + 2, :],
                            x8_sb[:, kd:kd + 2, ds(t0, TT)],
                            start=(kd == 0),
                            stop=(kd == KD - 2),
                            perf_mode=DR,
                        )
                    for kd in range(0, KD, 2):
                        nc.tensor.matmul(
                            v8[:],
                            w38_sb[:, kd:kd + 2, :],
                            x8_sb[:, kd:kd + 2, ds(t0, TT)],
                            start=(kd == 0),
                            stop=(kd == KD - 2),
                            perf_mode=DR,
                        )
                    s18 = gp.tile([P, TT], f32, tag="s18", name=f"s18_{t0}_{f}")
                    nc.scalar.activation(s18[:], u8[:], Silu, scale=1.0 / 32.0)
                    nc.vector.tensor_tensor(
                        h8_sb[:, fl, ds(t0, TT)], s18[:], v8[:], Alu.mult
                    )
                for t0, TT in tiles:
                    h1 = psA.tile([P, TT], f32, tag="h1", name=f"ph1_{t0}_{f}")
                    h3 = psA3.tile([P, TT], f32, tag="h3", name=f"ph3_{t0}_{f}")
                    for kd in range(KD):
                        nc.tensor.matmul(
                            h1[:],
                            w1_sb[:, kd, :],
                            x_sb[:, kd, ds(XO + t0, TT)],
                            start=(kd == 0),
                            stop=(kd == KD - 1),
                        )
                    for kd in range(KD):
                        nc.tensor.matmul(
                            h3[:],
                            w3_sb[:, kd, :],
                            x_sb[:, kd, ds(XO + t0, TT)],
                            start=(kd == 0),
                            stop=(kd == KD - 1),
                        )
                    s1 = gp.tile([P, TT], f32, tag="s1", name=f"s1_{t0}_{f}")
                    nc.scalar.activation(s1[:], h1[:], Silu)
                    nc.vector.tensor_mul(h_sb[:, fl, ds(t0, TT)], s1[:], h3[:])
                # remaining gating chains, then selector matmuls, one per slot
                if fh == 0:
                    if gpend:
                        emit_gating(*gpend.pop(0))
                    elif wrt_tiles:
                        emit_sel()

            if fh == 0:
                while gpend:
                    emit_gating(*gpend.pop(0))
                while wrt_tiles:
                    emit_sel()
                # wb/512 for the fp8 combine, one slice op
                nc.vector.tensor_scalar(
                    wb_lo[:], wb_all[:, ds(C_hi, C_lo + C_ov)],
                    1.0 / 512.0, None, Alu.mult,
                )

            # ---- phase B: yT(+=) (w2-half.T @ h) * wb ----
            for dm in range(KD):
                w2_sb = wp.tile([P, KH, P], bf16, tag="w2", name=f"w2_{fh}_{dm}")
                nc.sync.dma_start(w2_sb[:], w2[dm, :, ds(fh * KH, KH), :])
                w28_sb = wp8.tile([P, KH, P], f8, tag="w28", name=f"w28_{fh}_{dm}")
                nc.gpsimd.dma_start(w28_sb[:], w28[dm, :, ds(fh * KH, KH), :])
                w28o_sb = wp8.tile(
                    [P, KH, P], f8, tag="w28o", name=f"w28o_{fh}_{dm}"
                )
                nc.gpsimd.dma_start(w28o_sb[:], w28o[dm, :, ds(fh * KH, KH), :])
                dst = yt if fh == 0 else yt2
                # ov y chain
                for t0, TT in [(0, C_ov)]:
                    ypso = psB.tile(
                        [P, TT], f32, tag="y", name=f"yo_{t0}_{fh}_{dm}"
                    )
                    for fk in range(0, KH, 2):
                        nc.tensor.matmul(
                            ypso[:],
                            w28o_sb[:, fk:fk + 2, :],
                            h8_sb[:, fk:fk + 2, ds(C_lo + t0, TT)],
                            start=(fk == 0),
                            stop=(fk == KH - 2),
                            perf_mode=DR,
                        )
                    yo_sb = yp.tile(
                        [P, TT], bf16, tag="y_sb", name=f"yosb_{t0}_{fh}_{dm}"
                    )
                    nc.vector.tensor_mul(
                        yo_sb[:], ypso[:], wb_lo[:, ds(C_lo + t0, TT)]
                    )
                    nc.scalar.dma_start(dst[dm, :, ds(C + t0, TT)], yo_sb[:])
                # fp8 lo tiles: psum = 512*y; combine with wb/512
                for t0, TT in tiles8:
                    yps8 = psB.tile(
                        [P, TT], f32, tag="y", name=f"y8_{t0}_{fh}_{dm}"
                    )
                    for fk in range(0, KH, 2):
                        nc.tensor.matmul(
                            yps8[:],
                            w28_sb[:, fk:fk + 2, :],
                            h8_sb[:, fk:fk + 2, ds(t0, TT)],
                            start=(fk == 0),
                            stop=(fk == KH - 2),
                            perf_mode=DR,
                        )
                    y8_sb = yp.tile(
                        [P, TT], bf16, tag="y_sb", name=f"y8sb_{t0}_{fh}_{dm}"
                    )
                    nc.vector.tensor_mul(
                        y8_sb[:], yps8[:], wb_lo[:, ds(t0, TT)]
                    )
                    nc.scalar.dma_start(dst[dm, :, ds(C_hi + t0, TT)], y8_sb[:])

                for t0, TT in tiles:
                    yps = psB.tile([P, TT], f32, tag="y", name=f"y_{t0}_{fh}_{dm}")
                    for fk in range(KH):
                        nc.tensor.matmul(
                            yps[:],
                            w2_sb[:, fk, :],
                            h_sb[:, fk, ds(t0, TT)],
                            start=(fk == 0),
                            stop=(fk == KH - 1),
                        )
                    y_sb = yp.tile(
                        [P, TT], bf16, tag="y_sb", name=f"ysb_{t0}_{fh}_{dm}"
                    )
                    nc.vector.tensor_mul(y_sb[:], yps[:], wb_all[:, ds(t0, TT)])
                    nc.scalar.dma_start(dst[dm, :, ds(t0, TT)], y_sb[:])
    nc.compile()
    return nc


def _route(x: np.ndarray, gw: np.ndarray):
    """Top-2 expert selection + combine weights (host; softmax weights are
    used only for ORDERING tokens by weight — output values use the
    device-computed gate)."""
    logits = x @ gw
    n = x.shape[0]
    top1 = np.argmax(logits, axis=1)
    l2 = logits.copy()
    l2[np.arange(n), top1] = -np.inf
    top2 = np.argmax(l2, axis=1)
    m = np.maximum(logits[np.arange(n), top1], logits[np.arange(n), top2])
    e1 = np.exp(logits[np.arange(n), top1] - m)
    e2 = np.exp(logits[np.arange(n), top2] - m)
    cw1 = e1 / (e1 + e2)
    cw2 = e2 / (e1 + e2)
    idx, cws = [], []
    for e in range(gw.shape[1]):
        i1 = np.nonzero(top1 == e)[0]
        i2 = np.nonzero(top2 == e)[0]
        ii = np.concatenate([i1, i2])
        ww = np.concatenate([cw1[i1], cw2[i2]])
        order = np.argsort(ww)
        idx.append(ii[order].astype(np.int64))
        cws.append(ww[order])
    return idx


def _shuffle_w13(w: np.ndarray):
    # [D, F] -> [KF, P, KD, P] partition-major blocks
    KD, KF = D // P, F // P
    return np.ascontiguousarray(w.reshape(KD, P, KF, P).transpose(2, 1, 0, 3))


def _shuffle_w2(w: np.ndarray):
    # [F, D] -> [KD, P, KF, P]
    KD, KF = D // P, F // P
    return np.ascontiguousarray(w.reshape(KF, P, KD, P).transpose(2, 1, 0, 3))


def kernel(x, gate_w, w1, w2, w3, _trace=False, _trace_cores=None, _result_box=None):
    import ml_dtypes
    from concourse.bass_utils import run_bass_kernel_spmd

    bf16 = ml_dtypes.bfloat16
    f8 = ml_dtypes.float8_e4m3
    KD = D // P

    x = np.ascontiguousarray(np.asarray(x, dtype=np.float32))
    gw = np.ascontiguousarray(np.asarray(gate_w, dtype=np.float32))
    w1 = np.ascontiguousarray(np.asarray(w1, dtype=np.float32))
    w2 = np.ascontiguousarray(np.asarray(w2, dtype=np.float32))
    w3 = np.ascontiguousarray(np.asarray(w3, dtype=np.float32))
    assert x.shape == (T, D) and gw.shape == (D, E), (x.shape, gw.shape)
    assert w1.shape == (E, D, F) and w3.shape == (E, D, F), (w1.shape,)
    assert w2.shape == (E, F, D), (w2.shape,)

    idx = _route(x, gw)
    maxn = max(len(i) for i in idx)
    # own capacity: the mean load (padded); heavy experts ship their excess
    # (smallest-gate-weight pairs above the own-lo block) to other cores'
    # C_OV-wide fp8 overflow section. Falls back to max-load capacity if the
    # shipments don't fit.
    C = -(-(T * 2 // E) // P) * P
    ship = {e: len(idx[e]) - C for e in range(E) if len(idx[e]) > C}
    if sum(ship.values()) > E * C_OV:
        C = max(P, -(-maxn // P) * P)
        ship = {}
    C_lo = C_LO if all(len(i) > C_LO + ship.get(e, 0) for e, i in enumerate(idx)) else 0
    C_hi = C - C_lo
    # donor assignment: shipments split into <=C_OV pieces, each piece to
    # any core's ov slot (SPMD time is C+C_OV everywhere, so a donor may
    # host overflow too). assign[c] = (donor, start_within_shipment, n)
    assign = {}
    free = [c for c in range(E) if c not in ship] + sorted(ship)
    for d in sorted(ship, key=lambda e: -ship[e]):
        off = 0
        while off < ship[d]:
            n = min(C_OV, ship[d] - off)
            c = free.pop(0)
            assign[c] = (d, off, n)
            off += n
    for c in range(E):
        if c not in assign:
            assign[c] = (c, 0, 0)  # dummy: own weights, zero ov columns

    if (C, C_lo) not in _nc_cache:
        _nc_cache[(C, C_lo)] = _build(C, C_lo, C_OV)
    nc = _nc_cache[(C, C_lo)]

    rot = np.arange(E)
    XO = 2 * E
    # per-core pair lists: own-lo (C_lo smallest), shipped-out, own-hi
    los, his, ovs = [], [], []
    for e in range(E):
        k = ship.get(e, 0)
        los.append(idx[e][:C_lo])
        ovs.append(idx[e][C_lo:C_lo + k])      # shipped OUT of this expert
        his.append(idx[e][C_lo + k:])
    w18a = [_shuffle_w13((32.0 * w1[e]).astype(f8)) for e in range(E)]
    w38a = [_shuffle_w13((8.0 * w3[e]).astype(f8)) for e in range(E)]
    w28a = [_shuffle_w2((64.0 * w2[e]).astype(f8)) for e in range(E)]
    in_maps = []
    for e in range(E):
        lo, hi = los[e], his[e]
        n_hi = len(hi)
        d, ov_off, k = assign[e]
        ovt = ovs[d][ov_off:ov_off + k]
        xt = np.zeros((P, KD, XO + C + C_OV + ((-C_OV) % P)), bf16)
        gwr = np.ascontiguousarray(gw[:, (rot + e) % E]).astype(bf16)
        xt[:, :, :E] = gwr.reshape(KD, P, E).transpose(1, 0, 2)
        gwd = np.ascontiguousarray(gw[:, (rot + d) % E]).astype(bf16)
        xt[:, :, E:XO] = gwd.reshape(KD, P, E).transpose(1, 0, 2)
        xt[:, :, XO:XO + n_hi] = (
            x[hi].astype(bf16).reshape(n_hi, KD, P).transpose(2, 1, 0)
        )
        xt[:, :, XO + C_hi:XO + C] = (
            x[lo].astype(bf16).reshape(C_lo, KD, P).transpose(2, 1, 0)
        )
        if k:
            xt[:, :, XO + C:XO + C + k] = (
                x[ovt].astype(bf16).reshape(k, KD, P).transpose(2, 1, 0)
            )
        xt8 = np.zeros((P, KD, C_lo + C_OV), f8)
        xt8[:, :, :C_lo] = (
            x[lo].astype(f8).reshape(C_lo, KD, P).transpose(2, 1, 0)
        )
        if k:
            xt8[:, :, C_lo:C_lo + k] = (
                x[ovt].astype(f8).reshape(k, KD, P).transpose(2, 1, 0)
            )
        in_maps.append(
            {
                "xt": xt,
                "xt8": xt8,
                "w1": _shuffle_w13(w1[e].astype(bf16)),
                "w3": _shuffle_w13(w3[e].astype(bf16)),
                "w2": _shuffle_w2(w2[e].astype(bf16)),
                "w18": w18a[e],
                "w38": w38a[e],
                "w28": w28a[e],
                "w18o": w18a[d],
                "w38o": w38a[d],
                "w28o": w28a[d],
            }
        )

    res = run_bass_kernel_spmd(
        nc,
        in_maps,
        core_ids=list(range(NCORES)),
        trace=_trace,
        trace_cores=_trace_cores,
    )
    if _result_box is not None:
        _result_box.append(res)

    out = np.zeros((T, D), np.float32)
    for e in range(E):
        lo, hi = los[e], his[e]
        n_hi = len(hi)
        d, ov_off, k = assign[e]
        ovt = ovs[d][ov_off:ov_off + k]
        yt = np.asarray(res.results[e]["yt"])        # [KD, P, C + C_OV] bf16
        yt2 = np.asarray(res.results[e]["yt2"])
        ysum = yt.astype(np.float32) + yt2.astype(np.float32)
        out[hi] += ysum[:, :, :n_hi].reshape(D, n_hi).T
        if C_lo:
            out[lo] += ysum[:, :, C_hi:C].reshape(D, C_lo).T
        if k:
            out[ovt] += ysum[:, :, C:C + k].reshape(D, k).T
    return out
